# revision 38
# baseline (speedup 1.0000x reference)
"""Bahdanau additive attention kernel for Trainium2 (8 NeuronCores).

Problem shapes (hardcoded): B=4, Q=256, V=2048, H=512, U=128, fp32.

reference:
    pq = queries @ w1                  # [B,Q,U]
    pv = values  @ w2                  # [B,V,U]
    scores[b,q,v] = sum_u tanh(pq[b,q,u] + pv[b,v,u]) * v[u]
    attn = softmax(scores, axis=-1)
    out  = attn @ values               # [B,Q,H]

Sharding: 8 cores = 4 batches x 2 query-halves; full softmax per core,
no collectives.

Key idea: the 33.5M-per-core tanh evaluations (the baseline's ScalarE
roofline, ~190us) are replaced by a separable approximation
    tanh(s) ~= a1*s + a3*s^3 + a5*s^5 + sum_k beta_k sin(w_k s)
fit to max|err| 1.7e-3 over the empirical range |pq+pv| <= 8.35. Every
term factorizes over s = a + b:
    sin(w(a+b)) = sin(wa)cos(wb) + cos(wa)sin(wb)
    (a+b)^p     = sum_j C(p,j) a^(p-j) b^j
so scores become 15 PE matmul blocks of contraction dim U=128 against
[U,V] fp16 rhs tiles, and per-core transcendental work drops from Q*V*U
tanh to 2K*(Q+V)*U sin evals (~400x less). Pure-q terms are per-row
softmax shifts, folded into the exp bias (errors there cancel in the
softmax, so the whole q-bias path runs in fp16).

ACT Sin is accurate only on [-pi, pi] (no HW range reduction), so each
frequency's argument is range-reduced with an all-fp16 DVE chain (the
only DVE shapes that hit the 2x/4x perf modes; scalar_tensor_tensor is
always 1x, and GPSIMD is 15x slow AND starves DVE of SBUF ports):
    t  = ts(pv16 * (1/P) + 1536)   # fp16 magic-number round: t = 1536+m
    pm = ts((t - 1536) * -P)       # exact: P snapped to 8-bit mantissa
    xt = tt(pv16 + pm)             # xt in [-P/2, P/2] (+- fp16 ulp)
Then sin(w*xt) = sin(w*pv) (m wraps by whole periods, so fp16 slop in
the round is harmless), and cos(w*pv) = 1 - 2*sin^2(w/2*xt) with the
Sin(w/2) arg in [-pi/2, pi/2]; the square is a tt and the constant 1
drops into the exp bias. e2e sim of this pipeline: 9.0e-4.

Schedule notes: ~7us of SPMD prologue is fixed; DMA descriptor gen is
~700ns per dma_start serialized on its issuing sequencer, so inputs are
packed into few DMAs split across SP and ACT. All matmuls are fp16
(fp32 matmuls run half-rate LOW/HIGH passes). The pv16 PSUM->SBUF casts
and the pv^2/pv^4 squares run on ACT (Copy/Square share the Sin table)
during its ramp-up idle window; v-side work is processed in V/2 halves
behind the chunked valsT DMA + projection.
"""

from contextlib import ExitStack

import numpy as np

import concourse.bacc as bacc
import concourse.tile as tile
from concourse import mybir

B, Q, V, H, U = 4, 256, 2048, 512, 128
QL = Q // 2            # per-core queries
VT = V // 128          # 16 value tiles
HT = H // 128          # 4 hidden tiles

F32 = mybir.dt.float32
F16 = mybir.dt.float16

# tanh(s) ~= A1*s + A3*s^3 + A5*s^5 + sum_k BETA[k]*sin(2pi/P[k] * s);
# periods snapped to 8-bit mantissa so P*m is exact in fp16.
PS = [5.625, 3.515625, 2.5625, 1.65625, 2.015625]
A1, A3, A5 = 0.4617062370438008, -0.011904887078626084, 9.745956449752555e-05
BETAS = [0.2430037372439134, 0.08034949539217065, 0.028788466223929884,
         0.003511129873922167, 0.009955427280592441]
FREQS = [float(2 * np.pi / p) for p in PS]
K = len(FREQS)
NCOL = 6 + 2 * K
C16 = 1536.0           # fp16 round magic (1.5 * 2^10)

# consts16 packed layout: [w2 (HT*U) | beta_k c (K) | poly cols a1c,a3c,a5c
#                          | p5 outer (128) | identity (128)]
W2OFF = 0
BCOFF = HT * U
PCOFF = BCOFF + K
P5OFF = PCOFF + 3
IDOFF = P5OFF + 128
C16N = IDOFF + 128

SIN = mybir.ActivationFunctionType.Sin
EXP = mybir.ActivationFunctionType.Exp
SQUARE = mybir.ActivationFunctionType.Square
COPY = mybir.ActivationFunctionType.Copy
MULT = mybir.AluOpType.mult
ADD = mybir.AluOpType.add
SUB = mybir.AluOpType.subtract


def build_nc():
    nc = bacc.Bacc("TRN2", target_bir_lowering=False, debug=False)
    wq_ext = nc.declare_dram_parameter("wq16", [128, 2 * HT * 128], F16, isOutput=False)
    valsT_ext = nc.declare_dram_parameter("valsT16", [HT, 128, V], F16, isOutput=False)
    vals16_ext = nc.declare_dram_parameter("vals16", [VT, 128, H], F16, isOutput=False)
    cc_ext = nc.declare_dram_parameter("ccols", [128, NCOL], F32, isOutput=False)
    c16_ext = nc.declare_dram_parameter("consts16", [128, C16N], F16, isOutput=False)
    out_ext = nc.declare_dram_parameter("out", [QL, H], F32, isOutput=True)

    def tt(out, a, b, op):
        """Elementwise tensor-tensor on DVE (2x_1p perf mode for fp16)."""
        v = nc.vector
        return v.add_instruction(mybir.InstTensorTensor(
            name=nc.get_next_instruction_name(), op=op,
            ins=[v.lower_ap(a), v.lower_ap(b)], outs=[v.lower_ap(out)]))

    with tile.TileContext(nc) as tc, ExitStack() as ctx:
        singles = ctx.enter_context(tc.tile_pool(name="singles", bufs=1))
        work = ctx.enter_context(tc.tile_pool(name="work", bufs=3))
        xpool = ctx.enter_context(tc.tile_pool(name="xt", bufs=3))
        vpool = ctx.enter_context(tc.tile_pool(name="vtiles", bufs=2))

        # ---- input DMAs, split across the SP and ACT sequencers ----------
        sb_wq = singles.tile([128, 2 * HT * 128], F16)
        nc.sync.dma_start(out=sb_wq, in_=wq_ext[:])
        sb_valsT = singles.tile([128, HT, V], F16)
        for vc in range(3):
            vs = slice(vc * 512, (vc + 1) * 512)
            nc.sync.dma_start(out=sb_valsT[:, :, vs],
                              in_=valsT_ext.rearrange("t p v -> p t v")[:, :, vs])

        sb_c16 = singles.tile([128, C16N], F16)
        nc.scalar.dma_start(out=sb_c16, in_=c16_ext[:])
        sb_cc = singles.tile([128, NCOL], F32)
        nc.scalar.dma_start(out=sb_cc, in_=cc_ext[:])
        vs3 = slice(3 * 512, 4 * 512)
        nc.sync.dma_start(out=sb_valsT[:, :, vs3],
                          in_=valsT_ext.rearrange("t p v -> p t v")[:, :, vs3])
        sb_vals16 = singles.tile([128, VT, H], F16)
        nc.sync.dma_start(out=sb_vals16, in_=vals16_ext.rearrange("t p h -> p t h"))

        # Dummy 1-element Sin with no input deps: forces the trig ACT table
        # to load during the prologue idle window instead of injecting a
        # 1.28us ACT_TABLE_LOAD right before the first real sin.
        warm = work.tile([128, 1], F16, tag="warm")
        nc.vector.memset(warm, 0.5)
        warm2 = work.tile([128, 1], F16, tag="warm2")
        nc.scalar.activation(out=warm2, in_=warm, func=SIN, scale=1.0)

        def col(i):
            return sb_cc[:, i:i + 1]
        # 0:a1c 1:a3c 2:a5c 3:3a3c 4:10a5c 5:5a5c ; 6..: beta_k c ; 6+K..: -2 beta_k c
        C_A1, C_A3, C_A5, C_3A3, C_10A5, C_5A5 = range(6)
        sb_w1 = sb_wq[:, 0:HT * 128]
        sb_qTt = sb_wq[:, HT * 128:2 * HT * 128]
        sb_w2 = sb_c16[:, W2OFF:W2OFF + HT * U]
        sb_cc16 = sb_c16[:, BCOFF:BCOFF + K]
        sb_p5 = sb_c16[:, P5OFF:P5OFF + 128]
        identity16 = sb_c16[:, IDOFF:IDOFF + 128]

        # ---- pq projection (fp16): pqT [u, q] ----------------------------
        # pq16 lives as 128 extra columns of the pv16 tile: every v-side
        # chain pass / sin eval / square then processes the q-side for free
        # (the per-instruction overheads dominate small q-side ops).
        sb_pvq16 = singles.tile([128, V + QL], F16)
        sb_pq16 = sb_pvq16[:, V:V + QL]
        with tc.tile_pool(name="ps_pq", bufs=1, space="PSUM") as pqpool:
            ps_pq = pqpool.tile([128, QL], F32)
            for ht in range(HT):
                nc.tensor.matmul(ps_pq, lhsT=sb_w1[:, ht * U:(ht + 1) * U],
                                 rhs=sb_qTt[:, ht * QL:(ht + 1) * QL],
                                 start=(ht == 0), stop=(ht == HT - 1))
            nc.vector.tensor_copy(out=sb_pq16, in_=ps_pq)

        # pq powers + poly lhsT tiles (fp16; qbias precision is irrelevant,
        # it's a per-row softmax shift)
        sb_pq2 = singles.tile([128, QL], F16)
        tt(sb_pq2, sb_pq16, sb_pq16, MULT)
        sb_pq3 = singles.tile([128, QL], F16)
        tt(sb_pq3, sb_pq2, sb_pq16, MULT)
        sb_pq4 = singles.tile([128, QL], F16)
        tt(sb_pq4, sb_pq2, sb_pq2, MULT)
        sb_pq5 = singles.tile([128, QL], F16)
        tt(sb_pq5, sb_pq4, sb_pq16, MULT)

        lhsP1 = singles.tile([128, QL], F16)
        t1 = work.tile([128, QL], F32, tag="t1")
        nc.vector.tensor_scalar(t1, sb_pq4, col(C_5A5), col(C_A1), MULT, ADD)
        nc.vector.scalar_tensor_tensor(lhsP1, sb_pq2, col(C_3A3), t1, MULT, ADD)
        lhsP2 = singles.tile([128, QL], F16)
        t2 = work.tile([128, QL], F32, tag="t1")
        nc.vector.tensor_scalar(t2, sb_pq3, col(C_10A5), None, MULT)
        nc.vector.scalar_tensor_tensor(lhsP2, sb_pq16, col(C_3A3), t2, MULT, ADD)
        lhsP3 = singles.tile([128, QL], F16)
        nc.vector.tensor_scalar(lhsP3, sb_pq2, col(C_10A5), col(C_A3), MULT, ADD)
        lhsP4 = singles.tile([128, QL], F16)
        nc.vector.tensor_scalar(lhsP4, sb_pq16, col(C_5A5), None, MULT)

        # ---- main: pv projection, harmonics, scores ----------------------
        with tc.tile_pool(name="ps_scores", bufs=1, space="PSUM") as scpool:
            psum_scores = scpool.tile([128, V], F32)

            sb_pv16 = sb_pvq16[:, 0:V]
            with tc.tile_pool(name="ps_pv", bufs=1, space="PSUM") as pvpool:
                ps_pv = pvpool.tile([128, V], F32)
                for vc in range(4):
                    vs = slice(vc * 512, (vc + 1) * 512)
                    for ht in range(HT):
                        nc.tensor.matmul(ps_pv[:, vs],
                                         lhsT=sb_w2[:, ht * U:(ht + 1) * U],
                                         rhs=sb_valsT[:, ht, vs],
                                         start=(ht == 0), stop=(ht == HT - 1))
                    # PSUM->SBUF fp16 cast on ACT (Copy shares the Sin table)
                    nc.scalar.activation(out=sb_pv16[:, vs], in_=ps_pv[:, vs],
                                         func=COPY)

            with tc.tile_pool(name="ps_qb", bufs=1, space="PSUM") as qbpool:
                # q-bias: poly terms only. The pure-q sin terms are simply
                # dropped -- any per-row shift is softmax-invariant -- and a
                # constant -2 keeps exp() in fp16 range even at the absolute
                # worst case (|scores| <= 9.1, dropped sin terms <= 3.4).
                ps_qb = qbpool.tile([128, 1], F32)
                nc.tensor.matmul(ps_qb, lhsT=sb_pq16, rhs=sb_c16[:, PCOFF:PCOFF + 1],
                                 start=True, stop=False, skip_group_check=True)
                nc.tensor.matmul(ps_qb, lhsT=sb_pq3, rhs=sb_c16[:, PCOFF + 1:PCOFF + 2],
                                 start=False, stop=False, skip_group_check=True)
                nc.tensor.matmul(ps_qb, lhsT=sb_pq5, rhs=sb_c16[:, PCOFF + 2:PCOFF + 3],
                                 start=False, stop=True, skip_group_check=True)
                sb_qbias = singles.tile([128, 1], F32)
                nc.vector.tensor_scalar(sb_qbias, ps_qb, 1.0, -2.0, MULT, ADD)

            sb_pv2 = singles.tile([128, V], F16)
            sb_pv3 = singles.tile([128, V], F16)
            sb_pv4 = singles.tile([128, V], F16)
            sb_pv5 = singles.tile([128, V], F16)

            if True:
                nmm = 5 + 2 * K
                mmi = 0

                def score_mm(lhsT, rhs):
                    nonlocal mmi
                    for vc in range(4):
                        vs = slice(vc * 512, (vc + 1) * 512)
                        nc.tensor.matmul(psum_scores[:, vs], lhsT=lhsT,
                                         rhs=rhs[:, vs],
                                         start=(mmi == 0), stop=(mmi == nmm - 1),
                                         skip_group_check=True)
                    mmi += 1

                def score_mm2(lhsT_a, rhs_a, lhsT_b, rhs_b):
                    # chunk-major interleave of two blocks: the last chunk-c
                    # matmul lands as early as possible so the exp quarters
                    # (which need every block's chunk c) start sooner
                    nonlocal mmi
                    for vc in range(4):
                        vs = slice(vc * 512, (vc + 1) * 512)
                        nc.tensor.matmul(psum_scores[:, vs], lhsT=lhsT_a,
                                         rhs=rhs_a[:, vs],
                                         start=(mmi == 0), stop=False,
                                         skip_group_check=True)
                        nc.tensor.matmul(psum_scores[:, vs], lhsT=lhsT_b,
                                         rhs=rhs_b[:, vs],
                                         start=False, stop=(mmi == nmm - 2),
                                         skip_group_check=True)
                    mmi += 2

                def harmonic(k):
                    w, P = FREQS[k], PS[k]
                    sv = vpool.tile([128, V + QL], F16, tag="sv")
                    cvm = vpool.tile([128, V + QL], F16, tag="cvm")
                    # harmonic 0 chases the projection chunks; 1 runs in
                    # halves; the rest go full-width (less per-instruction
                    # overhead). The q-columns ride in the last span.
                    if k == 0:
                        spans = ((0, 512), (512, 1024), (1024, V + QL))
                    else:
                        spans = ((0, V + QL),)
                    for lo, hi in spans:
                        hs = slice(lo, hi)
                        n = hi - lo
                        tv = xpool.tile([128, n], F16, tag=f"tv{n}")
                        nc.vector.tensor_scalar(tv, sb_pvq16[:, hs], 1.0 / P, C16,
                                                MULT, ADD)
                        pmv = xpool.tile([128, n], F16, tag=f"pmv{n}")
                        nc.vector.tensor_scalar(pmv, tv, C16, -P, SUB, MULT)
                        xv = xpool.tile([128, n], F16, tag=f"xv{n}")
                        tt(xv, sb_pvq16[:, hs], pmv, ADD)
                        nc.scalar.activation(out=sv[:, hs], in_=xv, func=SIN, scale=w)
                        s2v = xpool.tile([128, n], F16, tag=f"s2v{n}")
                        nc.scalar.activation(out=s2v, in_=xv, func=SIN, scale=w / 2)
                        tt(cvm[:, hs], s2v, s2v, MULT)
                    # q-side lhsT tiles from the riding columns:
                    # sv[:, V:] = sin(w*pq), cvm[:, V:] = sin^2(w/2*pq)
                    la = work.tile([128, QL], F16, tag="la")
                    nc.vector.tensor_scalar(la, sv[:, V:V + QL],
                                            col(6 + K + k), None, MULT)
                    lb = work.tile([128, QL], F16, tag="lb")
                    nc.vector.tensor_scalar(lb, cvm[:, V:V + QL],
                                            col(6 + K + k), col(6 + k), MULT, ADD)
                    score_mm2(la, cvm, lb, sv)

                harmonic(0)
                harmonic(1)
                # power tiles + poly blocks mid-stream: issuing them before
                # the first harmonics would head-of-line-block ACT/DVE on the
                # full pv16
                nc.scalar.activation(out=sb_pv2, in_=sb_pv16, func=SQUARE)
                tt(sb_pv3, sb_pv2, sb_pv16, MULT)
                nc.scalar.activation(out=sb_pv4, in_=sb_pv2, func=SQUARE)
                tt(sb_pv5, sb_pv4, sb_pv16, MULT)
                score_mm(lhsP1, sb_pv16)
                score_mm(lhsP2, sb_pv2)
                score_mm(lhsP3, sb_pv3)
                score_mm(lhsP4, sb_pv4)
                score_mm(sb_p5, sb_pv5)
                for k in range(2, K):
                    harmonic(k)

            # ---- softmax + output, overlapped ----------------------------
            sb_e = singles.tile([128, V], F16)
            sb_sums = work.tile([128, 4], F32)
            with tc.tile_pool(name="ps_out", bufs=1, space="PSUM") as outpool, \
                    tc.tile_pool(name="ps_tr", bufs=2, space="PSUM") as trpool:
                ps_out = outpool.tile([128, H], F32, tag="ps_out")
                for c4 in range(4):
                    ks = slice(c4 * 512, (c4 + 1) * 512)
                    nc.scalar.activation(
                        out=sb_e[:, ks], in_=psum_scores[:, ks], func=EXP,
                        bias=sb_qbias[:, 0:1], scale=1.0,
                        accum_out=sb_sums[:, c4:c4 + 1])
                    ps_tr = trpool.tile([128, 512], F16, tag="ps_tr")
                    for j in range(4):
                        nc.tensor.transpose(
                            ps_tr[:, j * 128:(j + 1) * 128],
                            sb_e[:, (4 * c4 + j) * 128:(4 * c4 + j + 1) * 128],
                            identity16)
                    sb_eT = work.tile([128, 512], F16, tag="eT")
                    nc.vector.tensor_copy(out=sb_eT, in_=ps_tr)
                    for j in range(4):
                        vt = 4 * c4 + j
                        nc.tensor.matmul(
                            ps_out, lhsT=sb_eT[:, j * 128:(j + 1) * 128],
                            rhs=sb_vals16[:, vt, :],
                            start=(vt == 0), stop=(vt == VT - 1),
                            skip_group_check=True)
                sb_sum = work.tile([128, 1], F32)
                nc.vector.tensor_reduce(out=sb_sum, in_=sb_sums,
                                        axis=mybir.AxisListType.X,
                                        op=mybir.AluOpType.add)
                sb_rsum = work.tile([128, 1], F32)
                nc.vector.reciprocal(sb_rsum, sb_sum)
                sb_out = work.tile([128, H], F32)
                nc.vector.tensor_scalar_mul(sb_out, ps_out, sb_rsum)
                nc.sync.dma_start(out=out_ext[:], in_=sb_out)

    nc.finalize()
    return nc


_NC_CACHE = {}


def _get_nc():
    if "nc" not in _NC_CACHE:
        _NC_CACHE["nc"] = build_nc()
    return _NC_CACHE["nc"]


def make_in_maps(queries, values, w1, w2, v):
    queries = np.asarray(queries, np.float32)
    values = np.asarray(values, np.float32)
    c = np.asarray(v, np.float64)

    cols = np.zeros((128, NCOL), np.float32)
    cols[:, 0] = A1 * c
    cols[:, 1] = A3 * c
    cols[:, 2] = A5 * c
    cols[:, 3] = 3 * A3 * c
    cols[:, 4] = 10 * A5 * c
    cols[:, 5] = 5 * A5 * c
    for k in range(K):
        cols[:, 6 + k] = BETAS[k] * c
        cols[:, 6 + K + k] = -2 * BETAS[k] * c

    consts16 = np.zeros((128, C16N), np.float16)
    w2f = np.asarray(w2, np.float32).reshape(HT, 128, U)
    consts16[:, W2OFF:W2OFF + HT * U] = w2f.transpose(1, 0, 2).reshape(128, HT * U)
    consts16[:, BCOFF:BCOFF + K] = cols[:, 6:6 + K]
    consts16[:, PCOFF + 0] = A1 * c
    consts16[:, PCOFF + 1] = A3 * c
    consts16[:, PCOFF + 2] = A5 * c
    consts16[:, P5OFF:P5OFF + 128] = np.repeat((A5 * c)[:, None], 128, axis=1)
    consts16[:, IDOFF:IDOFF + 128] = np.eye(128)

    w1f = np.asarray(w1, np.float32).reshape(HT, 128, U)
    w1_16 = w1f.transpose(1, 0, 2).reshape(128, HT * U).astype(np.float16)

    in_maps = []
    for core in range(8):
        b, qh = core // 2, core % 2
        q_shard = queries[b, qh * QL:(qh + 1) * QL, :]        # [QL, H]
        qT = np.ascontiguousarray(q_shard.T).reshape(HT, 128, QL)
        wq = np.concatenate(
            [w1_16, qT.transpose(1, 0, 2).reshape(128, HT * QL).astype(np.float16)],
            axis=1)
        vb = values[b]                                        # [V, H]
        vbT16 = np.ascontiguousarray(vb.T.astype(np.float16)).reshape(HT, 128, V)
        in_maps.append({
            "wq16": np.ascontiguousarray(wq),
            "valsT16": vbT16,
            "vals16": np.ascontiguousarray(vb.astype(np.float16)).reshape(VT, 128, H),
            "ccols": cols, "consts16": consts16,
        })
    return in_maps


def gather_out(results):
    out = np.empty((B, Q, H), np.float32)
    for core in range(8):
        b, qh = core // 2, core % 2
        out[b, qh * QL:(qh + 1) * QL, :] = results[core]["out"]
    return out


def kernel(queries, values, w1, w2, v):
    from concourse.bass_utils import run_bass_kernel_spmd

    nc = _get_nc()
    in_maps = make_in_maps(queries, values, w1, w2, v)
    res = run_bass_kernel_spmd(nc, in_maps, list(range(8)))
    return gather_out(res.results)


# revision 40
# speedup vs baseline: 1.0147x; 1.0147x over previous
"""Bahdanau additive attention kernel for Trainium2 (8 NeuronCores).

Problem shapes (hardcoded): B=4, Q=256, V=2048, H=512, U=128, fp32.

reference:
    pq = queries @ w1                  # [B,Q,U]
    pv = values  @ w2                  # [B,V,U]
    scores[b,q,v] = sum_u tanh(pq[b,q,u] + pv[b,v,u]) * v[u]
    attn = softmax(scores, axis=-1)
    out  = attn @ values               # [B,Q,H]

Sharding: 8 cores = 4 batches x 2 query-halves; full softmax per core,
no collectives.

Key idea: the 33.5M-per-core tanh evaluations (the baseline's ScalarE
roofline, ~190us) are replaced by a separable approximation
    tanh(s) ~= a1*s + a3*s^3 + a5*s^5 + sum_k beta_k sin(w_k s)
fit to max|err| 1.7e-3 over the empirical range |pq+pv| <= 8.35. Every
term factorizes over s = a + b:
    sin(w(a+b)) = sin(wa)cos(wb) + cos(wa)sin(wb)
    (a+b)^p     = sum_j C(p,j) a^(p-j) b^j
so scores become 15 PE matmul blocks of contraction dim U=128 against
[U,V] fp16 rhs tiles, and per-core transcendental work drops from Q*V*U
tanh to 2K*(Q+V)*U sin evals (~400x less). Pure-q terms are per-row
softmax shifts, folded into the exp bias (errors there cancel in the
softmax, so the whole q-bias path runs in fp16).

ACT Sin is accurate only on [-pi, pi] (no HW range reduction), so each
frequency's argument is range-reduced with an all-fp16 DVE chain (the
only DVE shapes that hit the 2x/4x perf modes; scalar_tensor_tensor is
always 1x, and GPSIMD is 15x slow AND starves DVE of SBUF ports):
    t  = ts(pv16 * (1/P) + 1536)   # fp16 magic-number round: t = 1536+m
    pm = ts((t - 1536) * -P)       # exact: P snapped to 8-bit mantissa
    xt = tt(pv16 + pm)             # xt in [-P/2, P/2] (+- fp16 ulp)
Then sin(w*xt) = sin(w*pv) (m wraps by whole periods, so fp16 slop in
the round is harmless), and cos(w*pv) = 1 - 2*sin^2(w/2*xt) with the
Sin(w/2) arg in [-pi/2, pi/2]; the square is a tt and the constant 1
drops into the exp bias. e2e sim of this pipeline: 9.0e-4.

Schedule notes: ~7us of SPMD prologue is fixed; DMA descriptor gen is
~700ns per dma_start serialized on its issuing sequencer, so inputs are
packed into few DMAs split across SP and ACT. All matmuls are fp16
(fp32 matmuls run half-rate LOW/HIGH passes). The pv16 PSUM->SBUF casts
and the pv^2/pv^4 squares run on ACT (Copy/Square share the Sin table)
during its ramp-up idle window; v-side work is processed in V/2 halves
behind the chunked valsT DMA + projection.
"""

from contextlib import ExitStack

import numpy as np

import concourse.bacc as bacc
import concourse.tile as tile
from concourse import mybir

B, Q, V, H, U = 4, 256, 2048, 512, 128
QL = Q // 2            # per-core queries
VT = V // 128          # 16 value tiles
HT = H // 128          # 4 hidden tiles

F32 = mybir.dt.float32
F16 = mybir.dt.float16

# tanh(s) ~= A1*s + A3*s^3 + A5*s^5 + sum_k BETA[k]*sin(2pi/P[k] * s);
# periods snapped to 8-bit mantissa so P*m is exact in fp16.
PS = [5.625, 3.515625, 2.5625, 1.65625, 2.015625]
A1, A3, A5 = 0.4617062370438008, -0.011904887078626084, 9.745956449752555e-05
BETAS = [0.2430037372439134, 0.08034949539217065, 0.028788466223929884,
         0.003511129873922167, 0.009955427280592441]
FREQS = [float(2 * np.pi / p) for p in PS]
K = len(FREQS)
NCOL = 6 + 2 * K
C16 = 1536.0           # fp16 round magic (1.5 * 2^10)

# consts16 packed layout: [w2 (HT*U) | beta_k c (K) | poly cols a1c,a3c,a5c
#                          | p5 outer (128) | identity (128)]
W2OFF = 0
BCOFF = HT * U
PCOFF = BCOFF + K
P5OFF = PCOFF + 3
IDOFF = P5OFF + 128
C16N = IDOFF + 128

SIN = mybir.ActivationFunctionType.Sin
EXP = mybir.ActivationFunctionType.Exp
SQUARE = mybir.ActivationFunctionType.Square
COPY = mybir.ActivationFunctionType.Copy
MULT = mybir.AluOpType.mult
ADD = mybir.AluOpType.add
SUB = mybir.AluOpType.subtract


def build_nc():
    nc = bacc.Bacc("TRN2", target_bir_lowering=False, debug=False)
    wq_ext = nc.declare_dram_parameter("wq16", [128, 2 * HT * 128], F16, isOutput=False)
    valsT_ext = nc.declare_dram_parameter("valsT16", [HT, 128, V], F16, isOutput=False)
    vals16_ext = nc.declare_dram_parameter("vals16", [VT, 128, H], F16, isOutput=False)
    cc_ext = nc.declare_dram_parameter("ccols", [128, NCOL], F32, isOutput=False)
    c16_ext = nc.declare_dram_parameter("consts16", [128, C16N], F16, isOutput=False)
    out_ext = nc.declare_dram_parameter("out", [QL, H], F32, isOutput=True)

    def tt(out, a, b, op):
        """Elementwise tensor-tensor on DVE (2x_1p perf mode for fp16)."""
        v = nc.vector
        return v.add_instruction(mybir.InstTensorTensor(
            name=nc.get_next_instruction_name(), op=op,
            ins=[v.lower_ap(a), v.lower_ap(b)], outs=[v.lower_ap(out)]))

    with tile.TileContext(nc) as tc, ExitStack() as ctx:
        singles = ctx.enter_context(tc.tile_pool(name="singles", bufs=1))
        work = ctx.enter_context(tc.tile_pool(name="work", bufs=3))
        xpool = ctx.enter_context(tc.tile_pool(name="xt", bufs=3))
        vpool = ctx.enter_context(tc.tile_pool(name="vtiles", bufs=2))

        # ---- input DMAs, split across the SP and ACT sequencers ----------
        sb_wq = singles.tile([128, 2 * HT * 128], F16)
        nc.sync.dma_start(out=sb_wq, in_=wq_ext[:])
        sb_valsT = singles.tile([128, HT, V], F16)
        for vc in range(3):
            vs = slice(vc * 512, (vc + 1) * 512)
            nc.sync.dma_start(out=sb_valsT[:, :, vs],
                              in_=valsT_ext.rearrange("t p v -> p t v")[:, :, vs])

        sb_c16 = singles.tile([128, C16N], F16)
        nc.scalar.dma_start(out=sb_c16, in_=c16_ext[:])
        sb_cc = singles.tile([128, NCOL], F32)
        nc.scalar.dma_start(out=sb_cc, in_=cc_ext[:])
        vs3 = slice(3 * 512, 4 * 512)
        nc.sync.dma_start(out=sb_valsT[:, :, vs3],
                          in_=valsT_ext.rearrange("t p v -> p t v")[:, :, vs3])
        sb_vals16 = singles.tile([128, VT, H], F16)
        nc.sync.dma_start(out=sb_vals16, in_=vals16_ext.rearrange("t p h -> p t h"))

        # Dummy 1-element Sin with no input deps: forces the trig ACT table
        # to load during the prologue idle window instead of injecting a
        # 1.28us ACT_TABLE_LOAD right before the first real sin.
        warm = work.tile([128, 1], F16, tag="warm")
        nc.vector.memset(warm, 0.5)
        warm2 = work.tile([128, 1], F16, tag="warm2")
        nc.scalar.activation(out=warm2, in_=warm, func=SIN, scale=1.0)

        def col(i):
            return sb_cc[:, i:i + 1]
        # 0:a1c 1:a3c 2:a5c 3:3a3c 4:10a5c 5:5a5c ; 6..: beta_k c ; 6+K..: -2 beta_k c
        C_A1, C_A3, C_A5, C_3A3, C_10A5, C_5A5 = range(6)
        sb_w1 = sb_wq[:, 0:HT * 128]
        sb_qTt = sb_wq[:, HT * 128:2 * HT * 128]
        sb_w2 = sb_c16[:, W2OFF:W2OFF + HT * U]
        sb_cc16 = sb_c16[:, BCOFF:BCOFF + K]
        sb_p5 = sb_c16[:, P5OFF:P5OFF + 128]
        identity16 = sb_c16[:, IDOFF:IDOFF + 128]

        # ---- pq projection (fp16): pqT [u, q] ----------------------------
        # pq16 lives as 128 extra columns of the pv16 tile: every v-side
        # chain pass / sin eval / square then processes the q-side for free
        # (the per-instruction overheads dominate small q-side ops).
        sb_pvq16 = singles.tile([128, V + QL], F16)
        sb_pq16 = sb_pvq16[:, V:V + QL]
        with tc.tile_pool(name="ps_pq", bufs=1, space="PSUM") as pqpool:
            ps_pq = pqpool.tile([128, QL], F32)
            for ht in range(HT):
                nc.tensor.matmul(ps_pq, lhsT=sb_w1[:, ht * U:(ht + 1) * U],
                                 rhs=sb_qTt[:, ht * QL:(ht + 1) * QL],
                                 start=(ht == 0), stop=(ht == HT - 1))
            nc.vector.tensor_copy(out=sb_pq16, in_=ps_pq)

        # pq powers + poly lhsT tiles (fp16; qbias precision is irrelevant,
        # it's a per-row softmax shift)
        sb_pq2 = singles.tile([128, QL], F16)
        tt(sb_pq2, sb_pq16, sb_pq16, MULT)
        sb_pq3 = singles.tile([128, QL], F16)
        tt(sb_pq3, sb_pq2, sb_pq16, MULT)
        sb_pq4 = singles.tile([128, QL], F16)
        tt(sb_pq4, sb_pq2, sb_pq2, MULT)
        sb_pq5 = singles.tile([128, QL], F16)
        tt(sb_pq5, sb_pq4, sb_pq16, MULT)

        lhsP1 = singles.tile([128, QL], F16)
        t1 = work.tile([128, QL], F32, tag="t1")
        nc.vector.tensor_scalar(t1, sb_pq4, col(C_5A5), col(C_A1), MULT, ADD)
        nc.vector.scalar_tensor_tensor(lhsP1, sb_pq2, col(C_3A3), t1, MULT, ADD)
        lhsP2 = singles.tile([128, QL], F16)
        t2 = work.tile([128, QL], F32, tag="t1")
        nc.vector.tensor_scalar(t2, sb_pq3, col(C_10A5), None, MULT)
        nc.vector.scalar_tensor_tensor(lhsP2, sb_pq16, col(C_3A3), t2, MULT, ADD)
        lhsP3 = singles.tile([128, QL], F16)
        nc.vector.tensor_scalar(lhsP3, sb_pq2, col(C_10A5), col(C_A3), MULT, ADD)
        lhsP4 = singles.tile([128, QL], F16)
        nc.vector.tensor_scalar(lhsP4, sb_pq16, col(C_5A5), None, MULT)

        # ---- main: pv projection, harmonics, scores ----------------------
        with tc.tile_pool(name="ps_scores", bufs=1, space="PSUM") as scpool:
            psum_scores = scpool.tile([128, V], F32)

            sb_pv16 = sb_pvq16[:, 0:V]
            with tc.tile_pool(name="ps_pv", bufs=1, space="PSUM") as pvpool:
                ps_pv = pvpool.tile([128, V], F32)
                for vc in range(4):
                    vs = slice(vc * 512, (vc + 1) * 512)
                    for ht in range(HT):
                        nc.tensor.matmul(ps_pv[:, vs],
                                         lhsT=sb_w2[:, ht * U:(ht + 1) * U],
                                         rhs=sb_valsT[:, ht, vs],
                                         start=(ht == 0), stop=(ht == HT - 1))
                    # PSUM->SBUF fp16 cast on ACT (Copy shares the Sin table)
                    nc.scalar.activation(out=sb_pv16[:, vs], in_=ps_pv[:, vs],
                                         func=COPY)

            with tc.tile_pool(name="ps_qb", bufs=1, space="PSUM") as qbpool:
                # q-bias: poly terms only. The pure-q sin terms are simply
                # dropped -- any per-row shift is softmax-invariant -- and a
                # constant -2 keeps exp() in fp16 range even at the absolute
                # worst case (|scores| <= 9.1, dropped sin terms <= 3.4).
                ps_qb = qbpool.tile([128, 1], F32)
                nc.tensor.matmul(ps_qb, lhsT=sb_pq16, rhs=sb_c16[:, PCOFF:PCOFF + 1],
                                 start=True, stop=False, skip_group_check=True)
                nc.tensor.matmul(ps_qb, lhsT=sb_pq3, rhs=sb_c16[:, PCOFF + 1:PCOFF + 2],
                                 start=False, stop=False, skip_group_check=True)
                nc.tensor.matmul(ps_qb, lhsT=sb_pq5, rhs=sb_c16[:, PCOFF + 2:PCOFF + 3],
                                 start=False, stop=True, skip_group_check=True)
                sb_qbias = singles.tile([128, 1], F32)
                nc.vector.tensor_scalar(sb_qbias, ps_qb, 1.0, -2.0, MULT, ADD)

            sb_pv2 = singles.tile([128, V], F16)
            sb_pv3 = singles.tile([128, V], F16)
            sb_pv4 = singles.tile([128, V], F16)
            sb_pv5 = singles.tile([128, V], F16)

            if True:
                nmm = 5 + 2 * K
                mmi = 0

                def score_mm(lhsT, rhs):
                    nonlocal mmi
                    for vc in range(4):
                        vs = slice(vc * 512, (vc + 1) * 512)
                        nc.tensor.matmul(psum_scores[:, vs], lhsT=lhsT,
                                         rhs=rhs[:, vs],
                                         start=(mmi == 0), stop=(mmi == nmm - 1),
                                         skip_group_check=True)
                    mmi += 1

                def score_mm2(lhsT_a, rhs_a, lhsT_b, rhs_b):
                    # chunk-major interleave of two blocks: the last chunk-c
                    # matmul lands as early as possible so the exp quarters
                    # (which need every block's chunk c) start sooner
                    nonlocal mmi
                    for vc in range(4):
                        vs = slice(vc * 512, (vc + 1) * 512)
                        nc.tensor.matmul(psum_scores[:, vs], lhsT=lhsT_a,
                                         rhs=rhs_a[:, vs],
                                         start=(mmi == 0), stop=False,
                                         skip_group_check=True)
                        nc.tensor.matmul(psum_scores[:, vs], lhsT=lhsT_b,
                                         rhs=rhs_b[:, vs],
                                         start=False, stop=(mmi == nmm - 2),
                                         skip_group_check=True)
                    mmi += 2

                def harmonic(k):
                    w, P = FREQS[k], PS[k]
                    sv = vpool.tile([128, V + QL], F16, tag="sv")
                    cvm = vpool.tile([128, V + QL], F16, tag="cvm")
                    # harmonic 0 chases the projection chunks; 1 runs in
                    # halves; the rest go full-width (less per-instruction
                    # overhead). The q-columns ride in the last span.
                    if k == 0:
                        spans = ((0, 512), (512, 1024), (1024, V + QL))
                    else:
                        spans = ((0, V + QL),)
                    for lo, hi in spans:
                        hs = slice(lo, hi)
                        n = hi - lo
                        tv = xpool.tile([128, n], F16, tag=f"tv{n}")
                        nc.vector.tensor_scalar(tv, sb_pvq16[:, hs], 1.0 / P, C16,
                                                MULT, ADD)
                        pmv = xpool.tile([128, n], F16, tag=f"pmv{n}")
                        nc.vector.tensor_scalar(pmv, tv, C16, -P, SUB, MULT)
                        xv = xpool.tile([128, n], F16, tag=f"xv{n}")
                        tt(xv, sb_pvq16[:, hs], pmv, ADD)
                        nc.scalar.activation(out=sv[:, hs], in_=xv, func=SIN, scale=w)
                        s2v = xpool.tile([128, n], F16, tag=f"s2v{n}")
                        nc.scalar.activation(out=s2v, in_=xv, func=SIN, scale=w / 2)
                        tt(cvm[:, hs], s2v, s2v, MULT)
                    # q-side lhsT tiles from the riding columns:
                    # sv[:, V:] = sin(w*pq), cvm[:, V:] = sin^2(w/2*pq)
                    la = work.tile([128, QL], F16, tag="la")
                    nc.vector.tensor_scalar(la, sv[:, V:V + QL],
                                            col(6 + K + k), None, MULT)
                    lb = work.tile([128, QL], F16, tag="lb")
                    nc.vector.tensor_scalar(lb, cvm[:, V:V + QL],
                                            col(6 + K + k), col(6 + k), MULT, ADD)
                    score_mm2(la, cvm, lb, sv)

                harmonic(0)
                harmonic(1)
                # power tiles + poly blocks mid-stream: issuing them before
                # the first harmonics would head-of-line-block ACT/DVE on the
                # full pv16
                nc.scalar.activation(out=sb_pv2, in_=sb_pv16, func=SQUARE)
                tt(sb_pv3, sb_pv2, sb_pv16, MULT)
                nc.scalar.activation(out=sb_pv4, in_=sb_pv2, func=SQUARE)
                tt(sb_pv5, sb_pv4, sb_pv16, MULT)
                score_mm(lhsP1, sb_pv16)
                score_mm(lhsP2, sb_pv2)
                score_mm(lhsP3, sb_pv3)
                score_mm(lhsP4, sb_pv4)
                score_mm(sb_p5, sb_pv5)
                for k in range(2, K):
                    harmonic(k)

            # ---- softmax + output, overlapped ----------------------------
            sb_e = singles.tile([128, V], F16)
            with tc.tile_pool(name="ps_out", bufs=1, space="PSUM") as outpool, \
                    tc.tile_pool(name="ps_tr", bufs=2, space="PSUM") as trpool:
                ps_out = outpool.tile([128, H], F32, tag="ps_out")
                for c4 in range(4):
                    ks = slice(c4 * 512, (c4 + 1) * 512)
                    # no accum_out: the serialized ACTIVATION_READ_ACCUMULATOR
                    # between exp quarters costs ~1.2us of tail; the row sum
                    # is one DVE reduce issued after the eT copies, hidden
                    # under the final out matmuls
                    nc.scalar.activation(
                        out=sb_e[:, ks], in_=psum_scores[:, ks], func=EXP,
                        bias=sb_qbias[:, 0:1], scale=1.0)
                    ps_tr = trpool.tile([128, 512], F16, tag="ps_tr")
                    for j in range(4):
                        nc.tensor.transpose(
                            ps_tr[:, j * 128:(j + 1) * 128],
                            sb_e[:, (4 * c4 + j) * 128:(4 * c4 + j + 1) * 128],
                            identity16)
                    sb_eT = work.tile([128, 512], F16, tag="eT")
                    nc.vector.tensor_copy(out=sb_eT, in_=ps_tr)
                    for j in range(4):
                        vt = 4 * c4 + j
                        nc.tensor.matmul(
                            ps_out, lhsT=sb_eT[:, j * 128:(j + 1) * 128],
                            rhs=sb_vals16[:, vt, :],
                            start=(vt == 0), stop=(vt == VT - 1),
                            skip_group_check=True)
                sb_sum = work.tile([128, 1], F32)
                nc.vector.tensor_reduce(out=sb_sum, in_=sb_e,
                                        axis=mybir.AxisListType.X,
                                        op=mybir.AluOpType.add)
                sb_rsum = work.tile([128, 1], F32)
                nc.vector.reciprocal(sb_rsum, sb_sum)
                sb_out = work.tile([128, H], F32)
                nc.vector.tensor_scalar_mul(sb_out, ps_out, sb_rsum)
                nc.sync.dma_start(out=out_ext[:], in_=sb_out)

    nc.finalize()
    return nc


_NC_CACHE = {}


def _get_nc():
    if "nc" not in _NC_CACHE:
        _NC_CACHE["nc"] = build_nc()
    return _NC_CACHE["nc"]


def make_in_maps(queries, values, w1, w2, v):
    queries = np.asarray(queries, np.float32)
    values = np.asarray(values, np.float32)
    c = np.asarray(v, np.float64)

    cols = np.zeros((128, NCOL), np.float32)
    cols[:, 0] = A1 * c
    cols[:, 1] = A3 * c
    cols[:, 2] = A5 * c
    cols[:, 3] = 3 * A3 * c
    cols[:, 4] = 10 * A5 * c
    cols[:, 5] = 5 * A5 * c
    for k in range(K):
        cols[:, 6 + k] = BETAS[k] * c
        cols[:, 6 + K + k] = -2 * BETAS[k] * c

    consts16 = np.zeros((128, C16N), np.float16)
    w2f = np.asarray(w2, np.float32).reshape(HT, 128, U)
    consts16[:, W2OFF:W2OFF + HT * U] = w2f.transpose(1, 0, 2).reshape(128, HT * U)
    consts16[:, BCOFF:BCOFF + K] = cols[:, 6:6 + K]
    consts16[:, PCOFF + 0] = A1 * c
    consts16[:, PCOFF + 1] = A3 * c
    consts16[:, PCOFF + 2] = A5 * c
    consts16[:, P5OFF:P5OFF + 128] = np.repeat((A5 * c)[:, None], 128, axis=1)
    consts16[:, IDOFF:IDOFF + 128] = np.eye(128)

    w1f = np.asarray(w1, np.float32).reshape(HT, 128, U)
    w1_16 = w1f.transpose(1, 0, 2).reshape(128, HT * U).astype(np.float16)

    in_maps = []
    for core in range(8):
        b, qh = core // 2, core % 2
        q_shard = queries[b, qh * QL:(qh + 1) * QL, :]        # [QL, H]
        qT = np.ascontiguousarray(q_shard.T).reshape(HT, 128, QL)
        wq = np.concatenate(
            [w1_16, qT.transpose(1, 0, 2).reshape(128, HT * QL).astype(np.float16)],
            axis=1)
        vb = values[b]                                        # [V, H]
        vbT16 = np.ascontiguousarray(vb.T.astype(np.float16)).reshape(HT, 128, V)
        in_maps.append({
            "wq16": np.ascontiguousarray(wq),
            "valsT16": vbT16,
            "vals16": np.ascontiguousarray(vb.astype(np.float16)).reshape(VT, 128, H),
            "ccols": cols, "consts16": consts16,
        })
    return in_maps


def gather_out(results):
    out = np.empty((B, Q, H), np.float32)
    for core in range(8):
        b, qh = core // 2, core % 2
        out[b, qh * QL:(qh + 1) * QL, :] = results[core]["out"]
    return out


def kernel(queries, values, w1, w2, v):
    from concourse.bass_utils import run_bass_kernel_spmd

    nc = _get_nc()
    in_maps = make_in_maps(queries, values, w1, w2, v)
    res = run_bass_kernel_spmd(nc, in_maps, list(range(8)))
    return gather_out(res.results)


# revision 41
# speedup vs baseline: 1.2021x; 1.1847x over previous
"""Bahdanau additive attention kernel for Trainium2 (8 NeuronCores).

Problem shapes (hardcoded): B=4, Q=256, V=2048, H=512, U=128, fp32.

reference:
    pq = queries @ w1                  # [B,Q,U]
    pv = values  @ w2                  # [B,V,U]
    scores[b,q,v] = sum_u tanh(pq[b,q,u] + pv[b,v,u]) * v[u]
    attn = softmax(scores, axis=-1)
    out  = attn @ values               # [B,Q,H]

Sharding: 8 cores = 4 batches x 2 query-halves; full softmax per core,
no collectives.

Key idea: the 33.5M-per-core tanh evaluations (the baseline's ScalarE
roofline, ~190us) are replaced by a separable approximation
    tanh(s) ~= a1*s + a3*s^3 + a5*s^5 + sum_k beta_k sin(w_k s)
fit to max|err| 1.7e-3 over the empirical range |pq+pv| <= 8.35. Every
term factorizes over s = a + b:
    sin(w(a+b)) = sin(wa)cos(wb) + cos(wa)sin(wb)
    (a+b)^p     = sum_j C(p,j) a^(p-j) b^j
so scores become 15 PE matmul blocks of contraction dim U=128 against
[U,V] fp16 rhs tiles, and per-core transcendental work drops from Q*V*U
tanh to 2K*(Q+V)*U sin evals (~400x less). Pure-q terms are per-row
softmax shifts, folded into the exp bias (errors there cancel in the
softmax, so the whole q-bias path runs in fp16).

ACT Sin is accurate only on [-pi, pi] (no HW range reduction), so each
frequency's argument is range-reduced with an all-fp16 DVE chain (the
only DVE shapes that hit the 2x/4x perf modes; scalar_tensor_tensor is
always 1x, and GPSIMD is 15x slow AND starves DVE of SBUF ports):
    t  = ts(pv16 * (1/P) + 1536)   # fp16 magic-number round: t = 1536+m
    pm = ts((t - 1536) * -P)       # exact: P snapped to 8-bit mantissa
    xt = tt(pv16 + pm)             # xt in [-P/2, P/2] (+- fp16 ulp)
Then sin(w*xt) = sin(w*pv) (m wraps by whole periods, so fp16 slop in
the round is harmless), and cos(w*pv) = 1 - 2*sin^2(w/2*xt) with the
Sin(w/2) arg in [-pi/2, pi/2]; the square is a tt and the constant 1
drops into the exp bias. e2e sim of this pipeline: 9.0e-4.

Schedule notes: ~7us of SPMD prologue is fixed; DMA descriptor gen is
~700ns per dma_start serialized on its issuing sequencer, so inputs are
packed into few DMAs split across SP and ACT. All matmuls are fp16
(fp32 matmuls run half-rate LOW/HIGH passes). The pv16 PSUM->SBUF casts
and the pv^2/pv^4 squares run on ACT (Copy/Square share the Sin table)
during its ramp-up idle window; v-side work is processed in V/2 halves
behind the chunked valsT DMA + projection.
"""

from contextlib import ExitStack

import numpy as np

import concourse.bacc as bacc
import concourse.tile as tile
from concourse import mybir

B, Q, V, H, U = 4, 256, 2048, 512, 128
QL = Q // 2            # per-core queries
VT = V // 128          # 16 value tiles
HT = H // 128          # 4 hidden tiles

F32 = mybir.dt.float32
F16 = mybir.dt.float16

# tanh(s) ~= A1*s + A3*s^3 + A5*s^5 + sum_k BETA[k]*sin(2pi/P[k] * s);
# periods snapped to 8-bit mantissa so P*m is exact in fp16.
PS = [5.625, 3.515625, 2.5625, 1.65625, 2.015625]
A1, A3, A5 = 0.4617062370438008, -0.011904887078626084, 9.745956449752555e-05
BETAS = [0.2430037372439134, 0.08034949539217065, 0.028788466223929884,
         0.003511129873922167, 0.009955427280592441]
FREQS = [float(2 * np.pi / p) for p in PS]
K = len(FREQS)
NCOL = 6 + 2 * K
C16 = 1536.0           # fp16 round magic (1.5 * 2^10)

# consts16 packed layout: [w2 (HT*U) | beta_k c (K) | poly cols a1c,a3c,a5c
#                          | p5 outer (128) | identity (128)]
W2OFF = 0
BCOFF = HT * U
PCOFF = BCOFF + K
P5OFF = PCOFF + 3
IDOFF = P5OFF + 128
C16N = IDOFF + 128

SIN = mybir.ActivationFunctionType.Sin
EXP = mybir.ActivationFunctionType.Exp
SQUARE = mybir.ActivationFunctionType.Square
COPY = mybir.ActivationFunctionType.Copy
MULT = mybir.AluOpType.mult
ADD = mybir.AluOpType.add
SUB = mybir.AluOpType.subtract


def build_nc():
    nc = bacc.Bacc("TRN2", target_bir_lowering=False, debug=False)
    wq_ext = nc.declare_dram_parameter("wq16", [128, 2 * HT * 128], F16, isOutput=False)
    valsT_ext = nc.declare_dram_parameter("valsT16", [HT, 128, V], F16, isOutput=False)
    vals16_ext = nc.declare_dram_parameter("vals16", [VT, 128, H], F16, isOutput=False)
    cc_ext = nc.declare_dram_parameter("ccols", [128, NCOL], F32, isOutput=False)
    c16_ext = nc.declare_dram_parameter("consts16", [128, C16N], F16, isOutput=False)
    out_ext = nc.declare_dram_parameter("out", [QL, H], F32, isOutput=True)

    def tt(out, a, b, op):
        """Elementwise tensor-tensor on DVE (2x_1p perf mode for fp16)."""
        v = nc.vector
        return v.add_instruction(mybir.InstTensorTensor(
            name=nc.get_next_instruction_name(), op=op,
            ins=[v.lower_ap(a), v.lower_ap(b)], outs=[v.lower_ap(out)]))

    with tile.TileContext(nc) as tc, ExitStack() as ctx:
        singles = ctx.enter_context(tc.tile_pool(name="singles", bufs=1))
        work = ctx.enter_context(tc.tile_pool(name="work", bufs=3))
        xpool = ctx.enter_context(tc.tile_pool(name="xt", bufs=3))
        vpool = ctx.enter_context(tc.tile_pool(name="vtiles", bufs=2))

        # ---- input DMAs, split across the SP and ACT sequencers ----------
        sb_wq = singles.tile([128, 2 * HT * 128], F16)
        nc.sync.dma_start(out=sb_wq, in_=wq_ext[:])
        sb_valsT = singles.tile([128, HT, V], F16)
        for vc in range(3):
            vs = slice(vc * 512, (vc + 1) * 512)
            nc.sync.dma_start(out=sb_valsT[:, :, vs],
                              in_=valsT_ext.rearrange("t p v -> p t v")[:, :, vs])

        sb_c16 = singles.tile([128, C16N], F16)
        nc.scalar.dma_start(out=sb_c16, in_=c16_ext[:])
        sb_cc = singles.tile([128, NCOL], F32)
        nc.scalar.dma_start(out=sb_cc, in_=cc_ext[:])
        vs3 = slice(3 * 512, 4 * 512)
        nc.sync.dma_start(out=sb_valsT[:, :, vs3],
                          in_=valsT_ext.rearrange("t p v -> p t v")[:, :, vs3])
        sb_vals16 = singles.tile([128, VT, H], F16)
        nc.sync.dma_start(out=sb_vals16, in_=vals16_ext.rearrange("t p h -> p t h"))

        # Dummy 1-element Sin with no input deps: forces the trig ACT table
        # to load during the prologue idle window instead of injecting a
        # 1.28us ACT_TABLE_LOAD right before the first real sin.
        warm = work.tile([128, 1], F16, tag="warm")
        nc.vector.memset(warm, 0.5)
        warm2 = work.tile([128, 1], F16, tag="warm2")
        nc.scalar.activation(out=warm2, in_=warm, func=SIN, scale=1.0)

        def col(i):
            return sb_cc[:, i:i + 1]
        # 0:a1c 1:a3c 2:a5c 3:3a3c 4:10a5c 5:5a5c ; 6..: beta_k c ; 6+K..: -2 beta_k c
        C_A1, C_A3, C_A5, C_3A3, C_10A5, C_5A5 = range(6)
        sb_w1 = sb_wq[:, 0:HT * 128]
        sb_qTt = sb_wq[:, HT * 128:2 * HT * 128]
        sb_w2 = sb_c16[:, W2OFF:W2OFF + HT * U]
        sb_cc16 = sb_c16[:, BCOFF:BCOFF + K]
        sb_p5 = sb_c16[:, P5OFF:P5OFF + 128]
        identity16 = sb_c16[:, IDOFF:IDOFF + 128]

        # ---- pq projection (fp16): pqT [u, q] ----------------------------
        # pq16 lives as 128 extra columns of the pv16 tile: every v-side
        # chain pass / sin eval / square then processes the q-side for free
        # (the per-instruction overheads dominate small q-side ops).
        sb_pvq16 = singles.tile([128, V + QL], F16)
        sb_pq16 = sb_pvq16[:, V:V + QL]
        with tc.tile_pool(name="ps_pq", bufs=1, space="PSUM") as pqpool:
            ps_pq = pqpool.tile([128, QL], F32)
            for ht in range(HT):
                nc.tensor.matmul(ps_pq, lhsT=sb_w1[:, ht * U:(ht + 1) * U],
                                 rhs=sb_qTt[:, ht * QL:(ht + 1) * QL],
                                 start=(ht == 0), stop=(ht == HT - 1))
            nc.vector.tensor_copy(out=sb_pq16, in_=ps_pq)

        # pq powers + poly lhsT tiles (fp16; qbias precision is irrelevant,
        # it's a per-row softmax shift)
        sb_pq2 = singles.tile([128, QL], F16)
        tt(sb_pq2, sb_pq16, sb_pq16, MULT)
        sb_pq3 = singles.tile([128, QL], F16)
        tt(sb_pq3, sb_pq2, sb_pq16, MULT)
        sb_pq4 = singles.tile([128, QL], F16)
        tt(sb_pq4, sb_pq2, sb_pq2, MULT)
        sb_pq5 = singles.tile([128, QL], F16)
        tt(sb_pq5, sb_pq4, sb_pq16, MULT)

        lhsP1 = singles.tile([128, QL], F16)
        t1 = work.tile([128, QL], F32, tag="t1")
        nc.vector.tensor_scalar(t1, sb_pq4, col(C_5A5), col(C_A1), MULT, ADD)
        nc.vector.scalar_tensor_tensor(lhsP1, sb_pq2, col(C_3A3), t1, MULT, ADD)
        lhsP2 = singles.tile([128, QL], F16)
        t2 = work.tile([128, QL], F32, tag="t1")
        nc.vector.tensor_scalar(t2, sb_pq3, col(C_10A5), None, MULT)
        nc.vector.scalar_tensor_tensor(lhsP2, sb_pq16, col(C_3A3), t2, MULT, ADD)
        lhsP3 = singles.tile([128, QL], F16)
        nc.vector.tensor_scalar(lhsP3, sb_pq2, col(C_10A5), col(C_A3), MULT, ADD)
        lhsP4 = singles.tile([128, QL], F16)
        nc.vector.tensor_scalar(lhsP4, sb_pq16, col(C_5A5), None, MULT)

        # ---- main: pv projection, harmonics, scores ----------------------
        with tc.tile_pool(name="ps_scores", bufs=1, space="PSUM") as scpool:
            psum_scores = scpool.tile([128, V], F32)

            sb_pv16 = sb_pvq16[:, 0:V]
            with tc.tile_pool(name="ps_pv", bufs=1, space="PSUM") as pvpool:
                ps_pv = pvpool.tile([128, V], F32)
                for vc in range(4):
                    vs = slice(vc * 512, (vc + 1) * 512)
                    for ht in range(HT):
                        nc.tensor.matmul(ps_pv[:, vs],
                                         lhsT=sb_w2[:, ht * U:(ht + 1) * U],
                                         rhs=sb_valsT[:, ht, vs],
                                         start=(ht == 0), stop=(ht == HT - 1))
                    # PSUM->SBUF fp16 cast on ACT (Copy shares the Sin table)
                    nc.scalar.activation(out=sb_pv16[:, vs], in_=ps_pv[:, vs],
                                         func=COPY)

            with tc.tile_pool(name="ps_qb", bufs=1, space="PSUM") as qbpool:
                # q-bias: poly terms only. The pure-q sin terms are simply
                # dropped -- any per-row shift is softmax-invariant -- and a
                # constant -2 keeps exp() in fp16 range even at the absolute
                # worst case (|scores| <= 9.1, dropped sin terms <= 3.4).
                ps_qb = qbpool.tile([128, 1], F32)
                nc.tensor.matmul(ps_qb, lhsT=sb_pq16, rhs=sb_c16[:, PCOFF:PCOFF + 1],
                                 start=True, stop=False, skip_group_check=True)
                nc.tensor.matmul(ps_qb, lhsT=sb_pq3, rhs=sb_c16[:, PCOFF + 1:PCOFF + 2],
                                 start=False, stop=False, skip_group_check=True)
                nc.tensor.matmul(ps_qb, lhsT=sb_pq5, rhs=sb_c16[:, PCOFF + 2:PCOFF + 3],
                                 start=False, stop=True, skip_group_check=True)
                sb_qbias = singles.tile([128, 1], F32)
                nc.vector.tensor_scalar(sb_qbias, ps_qb, 1.0, -2.0, MULT, ADD)

            sb_pv2 = singles.tile([128, V], F16)
            sb_pv3 = singles.tile([128, V], F16)
            sb_pv4 = singles.tile([128, V], F16)
            sb_pv5 = singles.tile([128, V], F16)

            if True:
                nmm = 5 + 2 * K
                mmi = 0

                def score_mm(lhsT, rhs):
                    nonlocal mmi
                    for vc in range(4):
                        vs = slice(vc * 512, (vc + 1) * 512)
                        nc.tensor.matmul(psum_scores[:, vs], lhsT=lhsT,
                                         rhs=rhs[:, vs],
                                         start=(mmi == 0), stop=(mmi == nmm - 1),
                                         skip_group_check=True)
                    mmi += 1

                def score_mm2(lhsT_a, rhs_a, lhsT_b, rhs_b):
                    # chunk-major interleave of two blocks: the last chunk-c
                    # matmul lands as early as possible so the exp quarters
                    # (which need every block's chunk c) start sooner
                    nonlocal mmi
                    for vc in range(4):
                        vs = slice(vc * 512, (vc + 1) * 512)
                        nc.tensor.matmul(psum_scores[:, vs], lhsT=lhsT_a,
                                         rhs=rhs_a[:, vs],
                                         start=(mmi == 0), stop=False,
                                         skip_group_check=True)
                        nc.tensor.matmul(psum_scores[:, vs], lhsT=lhsT_b,
                                         rhs=rhs_b[:, vs],
                                         start=False, stop=(mmi == nmm - 2),
                                         skip_group_check=True)
                    mmi += 2

                def harmonic(k):
                    w, P = FREQS[k], PS[k]
                    sv = vpool.tile([128, V + QL], F16, tag="sv")
                    cvm = vpool.tile([128, V + QL], F16, tag="cvm")
                    # harmonic 0 chases the projection chunks; 1 runs in
                    # halves; the rest go full-width (less per-instruction
                    # overhead). The q-columns ride in the last span.
                    if k == 0:
                        spans = ((0, 512), (512, 1024), (1024, V + QL))
                    else:
                        spans = ((0, V + QL),)
                    for lo, hi in spans:
                        hs = slice(lo, hi)
                        n = hi - lo
                        tv = xpool.tile([128, n], F16, tag=f"tv{n}")
                        nc.vector.tensor_scalar(tv, sb_pvq16[:, hs], 1.0 / P, C16,
                                                MULT, ADD)
                        pmv = xpool.tile([128, n], F16, tag=f"pmv{n}")
                        nc.vector.tensor_scalar(pmv, tv, C16, -P, SUB, MULT)
                        xv = xpool.tile([128, n], F16, tag=f"xv{n}")
                        tt(xv, sb_pvq16[:, hs], pmv, ADD)
                        nc.scalar.activation(out=sv[:, hs], in_=xv, func=SIN, scale=w)
                        s2v = xpool.tile([128, n], F16, tag=f"s2v{n}")
                        nc.scalar.activation(out=s2v, in_=xv, func=SIN, scale=w / 2)
                        tt(cvm[:, hs], s2v, s2v, MULT)
                    # q-side lhsT tiles from the riding columns:
                    # sv[:, V:] = sin(w*pq), cvm[:, V:] = sin^2(w/2*pq)
                    la = work.tile([128, QL], F16, tag="la")
                    nc.vector.tensor_scalar(la, sv[:, V:V + QL],
                                            col(6 + K + k), None, MULT)
                    lb = work.tile([128, QL], F16, tag="lb")
                    nc.vector.tensor_scalar(lb, cvm[:, V:V + QL],
                                            col(6 + K + k), col(6 + k), MULT, ADD)
                    score_mm2(la, cvm, lb, sv)

                harmonic(0)
                harmonic(1)
                # power tiles + poly blocks mid-stream: issuing them before
                # the first harmonics would head-of-line-block ACT/DVE on the
                # full pv16
                nc.scalar.activation(out=sb_pv2, in_=sb_pv16, func=SQUARE)
                tt(sb_pv3, sb_pv2, sb_pv16, MULT)
                nc.scalar.activation(out=sb_pv4, in_=sb_pv2, func=SQUARE)
                tt(sb_pv5, sb_pv4, sb_pv16, MULT)
                score_mm(lhsP1, sb_pv16)
                score_mm(lhsP2, sb_pv2)
                score_mm(lhsP3, sb_pv3)
                score_mm(lhsP4, sb_pv4)
                score_mm(sb_p5, sb_pv5)
                for k in range(2, K):
                    harmonic(k)

            # ---- softmax + output, overlapped ----------------------------
            sb_e = singles.tile([128, V], F16)
            with tc.tile_pool(name="ps_out", bufs=1, space="PSUM") as outpool, \
                    tc.tile_pool(name="ps_tr", bufs=3, space="PSUM") as trpool:
                ps_out = outpool.tile([128, H], F32, tag="ps_out")
                for c4 in range(4):
                    ks = slice(c4 * 512, (c4 + 1) * 512)
                    # no accum_out: the serialized ACTIVATION_READ_ACCUMULATOR
                    # between exp quarters costs ~1.2us of tail; the row sum
                    # is one DVE reduce issued after the eT copies, hidden
                    # under the final out matmuls
                    nc.scalar.activation(
                        out=sb_e[:, ks], in_=psum_scores[:, ks], func=EXP,
                        bias=sb_qbias[:, 0:1], scale=1.0)
                    ps_tr = trpool.tile([128, 512], F16, tag="ps_tr")
                    for j in range(4):
                        nc.tensor.transpose(
                            ps_tr[:, j * 128:(j + 1) * 128],
                            sb_e[:, (4 * c4 + j) * 128:(4 * c4 + j + 1) * 128],
                            identity16)
                    sb_eT = work.tile([128, 512], F16, tag="eT")
                    nc.vector.tensor_copy(out=sb_eT, in_=ps_tr)
                    for j in range(4):
                        vt = 4 * c4 + j
                        nc.tensor.matmul(
                            ps_out, lhsT=sb_eT[:, j * 128:(j + 1) * 128],
                            rhs=sb_vals16[:, vt, :],
                            start=(vt == 0), stop=(vt == VT - 1),
                            skip_group_check=True)
                sb_sum = work.tile([128, 1], F32)
                nc.vector.tensor_reduce(out=sb_sum, in_=sb_e,
                                        axis=mybir.AxisListType.X,
                                        op=mybir.AluOpType.add)
                sb_rsum = work.tile([128, 1], F32)
                nc.vector.reciprocal(sb_rsum, sb_sum)
                sb_out = work.tile([128, H], F32)
                nc.vector.tensor_scalar_mul(sb_out, ps_out, sb_rsum)
                nc.sync.dma_start(out=out_ext[:], in_=sb_out)

    nc.finalize()
    return nc


_NC_CACHE = {}


def _get_nc():
    if "nc" not in _NC_CACHE:
        _NC_CACHE["nc"] = build_nc()
    return _NC_CACHE["nc"]


def make_in_maps(queries, values, w1, w2, v):
    queries = np.asarray(queries, np.float32)
    values = np.asarray(values, np.float32)
    c = np.asarray(v, np.float64)

    cols = np.zeros((128, NCOL), np.float32)
    cols[:, 0] = A1 * c
    cols[:, 1] = A3 * c
    cols[:, 2] = A5 * c
    cols[:, 3] = 3 * A3 * c
    cols[:, 4] = 10 * A5 * c
    cols[:, 5] = 5 * A5 * c
    for k in range(K):
        cols[:, 6 + k] = BETAS[k] * c
        cols[:, 6 + K + k] = -2 * BETAS[k] * c

    consts16 = np.zeros((128, C16N), np.float16)
    w2f = np.asarray(w2, np.float32).reshape(HT, 128, U)
    consts16[:, W2OFF:W2OFF + HT * U] = w2f.transpose(1, 0, 2).reshape(128, HT * U)
    consts16[:, BCOFF:BCOFF + K] = cols[:, 6:6 + K]
    consts16[:, PCOFF + 0] = A1 * c
    consts16[:, PCOFF + 1] = A3 * c
    consts16[:, PCOFF + 2] = A5 * c
    consts16[:, P5OFF:P5OFF + 128] = np.repeat((A5 * c)[:, None], 128, axis=1)
    consts16[:, IDOFF:IDOFF + 128] = np.eye(128)

    w1f = np.asarray(w1, np.float32).reshape(HT, 128, U)
    w1_16 = w1f.transpose(1, 0, 2).reshape(128, HT * U).astype(np.float16)

    in_maps = []
    for core in range(8):
        b, qh = core // 2, core % 2
        q_shard = queries[b, qh * QL:(qh + 1) * QL, :]        # [QL, H]
        qT = np.ascontiguousarray(q_shard.T).reshape(HT, 128, QL)
        wq = np.concatenate(
            [w1_16, qT.transpose(1, 0, 2).reshape(128, HT * QL).astype(np.float16)],
            axis=1)
        vb = values[b]                                        # [V, H]
        vbT16 = np.ascontiguousarray(vb.T.astype(np.float16)).reshape(HT, 128, V)
        in_maps.append({
            "wq16": np.ascontiguousarray(wq),
            "valsT16": vbT16,
            "vals16": np.ascontiguousarray(vb.astype(np.float16)).reshape(VT, 128, H),
            "ccols": cols, "consts16": consts16,
        })
    return in_maps


def gather_out(results):
    out = np.empty((B, Q, H), np.float32)
    for core in range(8):
        b, qh = core // 2, core % 2
        out[b, qh * QL:(qh + 1) * QL, :] = results[core]["out"]
    return out


def kernel(queries, values, w1, w2, v):
    from concourse.bass_utils import run_bass_kernel_spmd

    nc = _get_nc()
    in_maps = make_in_maps(queries, values, w1, w2, v)
    res = run_bass_kernel_spmd(nc, in_maps, list(range(8)))
    return gather_out(res.results)
